# revision 16
# baseline (speedup 1.0000x reference)
"""Trainium2 Bass kernel for the CIFAR10 Monarch MLP (7 monarch layers + log_softmax).

Strategy
--------
Pure data parallel over 8 NeuronCores: each core takes a 1024-row batch shard;
the ~9M-param block-diagonal weights are replicated.

On-device dataflow is feature-major: activations live in SBUF as
[features (128-partition tiles), batch (free dim)].  The monarch butterfly
permutation is folded entirely into a host-side rearrangement of the weights:

  GEMM1:  W1cols[p_local, midcol(l,r)] = w1[k, q, p]   with j=4r+l, k=j//Q, q=j%Q
  GEMM2:  W2cols[r_local, outcol(l,s)] = w2[l, s, r]

so both GEMMs become plain matmuls over contiguous feature tiles; no on-device
transpose or permutation is needed anywhere except one tiny 12xN PE transpose
feeding the final log_softmax.

Matmul outputs must start at a 32-aligned PSUM partition, so each 128-row
output tile is produced by a single accumulation chain covering rows [0, valid)
with per-(tile, input-block) column-masked weight slices built on the host
(masking costs nothing on the PE: matmul time scales with N, not M).

Layers run one at a time over the full per-core batch (2 slices of 512
columns); intermediate activations round-trip DRAM between layers.  Matmuls
use the fp32r PE path (fp32 storage, full rate at N=512).
"""

import numpy as np

import concourse.bass as bass
import concourse.bacc as bacc_mod
import concourse.mybir as mybir
import concourse.tile as tile
from concourse.bass_utils import run_bass_kernel_spmd
from concourse.masks import make_identity

# ----------------------------------------------------------------- problem dims
BATCH = 8192
IN_FEATURES = 3072
NCORES = 8
BPC = BATCH // NCORES          # 1024 batch rows per core
NSL = 2                        # batch slices per core
SLICE = BPC // NSL             # 512
NOUT = 10

SHAPES = [((4, 750, 768), (4, 750, 750)),
          ((4, 500, 750), (4, 500, 500)),
          ((4, 250, 500), (4, 250, 250)),
          ((4, 125, 250), (4, 125, 125)),
          ((4, 50, 125), (4, 50, 50)),
          ((4, 25, 50), (4, 25, 25)),
          ((4, 3, 25), (4, 3, 3))]
NLAYERS = 7

F32 = mybir.dt.float32
# Activation/weight chain dtype for the two GEMMs: "fp32" (exact, 4 cyc/row),
# "fp32r" (fp32 storage, ~tf32 matmul precision, 1 cyc/row at N=512), or
# "bf16" (half storage/traffic, 1 cyc/row).
import os as _os
ACT_DT_NAME = _os.environ.get("KERNEL_MM_DT", "fp32r")
ACT_DT = {"fp32": mybir.dt.float32,
          "fp32r": mybir.dt.float32r,
          "bf16": mybir.dt.bfloat16}[ACT_DT_NAME]


# ------------------------------------------------------------------ layouts
def make_layout(Sb, pack=False):
    """4 blocks of size Sb in 128-row tiles. Non-final layouts give every
    block its own tile at offset 0 (HW: accumulation chains cannot mix
    tile_position row bases, so contraction operands must sit at base 0).
    pack=True (final-layer output only) packs blocks at 32-aligned offsets
    inside one tile; those offsets only appear on the masked M side.
    Returns (ntiles, chunks), chunks[k] = [(tile, off, ln), ...]"""
    if Sb >= 128:
        cpb = (Sb + 127) // 128
        chunks = []
        for k in range(4):
            ck, rem, c = [], Sb, 0
            while rem > 0:
                ln = min(128, rem)
                ck.append((k * cpb + c, 0, ln))
                rem -= ln
                c += 1
            chunks.append(ck)
        return 4 * cpb, chunks
    if pack:
        stride = ((Sb + 31) // 32) * 32
        bpt = max(1, 128 // stride)
        ntiles = (4 + bpt - 1) // bpt
        return ntiles, [[(k // bpt, (k % bpt) * stride, Sb)] for k in range(4)]
    return 4, [[(k, 0, Sb)] for k in range(4)]


def cols_of_block(chunks, k):
    out = []
    for (t, off, ln) in chunks[k]:
        out.extend(range(t * 128 + off, t * 128 + off + ln))
    return np.array(out, dtype=np.int64)


class LayerPlan:
    def __init__(self, li, w1_shape, w2_shape):
        _, Q, P = w1_shape
        _, S, R = w2_shape
        self.li, self.P, self.Q, self.R, self.S = li, P, Q, R, S
        self.in_tiles, self.in_chunks = make_layout(P)
        self.mid_tiles, self.mid_chunks = make_layout(R)
        self.out_tiles, self.out_chunks = make_layout(S, pack=(li == NLAYERS - 1))
        self.nin_cpb = len(self.in_chunks[0])
        self.nmid_cpb = len(self.mid_chunks[0])
        self.w1_rows = self.nin_cpb * 128
        self.w2_rows = self.nmid_cpb * 128
        self.ngroups = 2 if li <= 1 else 1
        self._build_schedules()

    def _build_schedules(self):
        # g1 segments (pre-masking): (mid_tile, row_a, row_b, colA, colB, k)
        segs = []
        Q = self.Q
        for l in range(4):
            r0 = 0
            for (t, off, ln) in self.mid_chunks[l]:
                rr = np.arange(r0, r0 + ln)
                ks = (4 * rr + l) // Q
                a = 0
                while a < ln:
                    k, b = ks[a], a
                    while b < ln and ks[b] == k:
                        b += 1
                    segs.append((t, off + a, off + b,
                                 t * 128 + off + a, t * 128 + off + b, int(k)))
                    a = b
                r0 += ln
        self.g1_segs = segs
        # masked weight-1 blocks: one [w1_rows, 128] column block per (tile, k)
        self.w1_blocks = []        # list of (tile, k)
        self.w1_block_of = {}      # (tile, k) -> index
        for (t, ra, rb, ca, cb, k) in segs:
            if (t, k) not in self.w1_block_of:
                self.w1_block_of[(t, k)] = len(self.w1_blocks)
                self.w1_blocks.append((t, k))
        self.mid_valid = {}
        for (t, ra, rb, _, _, _) in segs:
            self.mid_valid[t] = max(self.mid_valid.get(t, 0), rb)
        # g1 schedule per mid tile: ordered (k, block_idx)
        self.g1_by_tile = {}
        for (t, k) in self.w1_blocks:
            self.g1_by_tile.setdefault(t, []).append((k, self.w1_block_of[(t, k)]))

        # g2 groups: (out_tile, row_a, row_b, ocolA, ocolB, l)
        self.g2_grps = [(t, off, off + ln, t * 128 + off, t * 128 + off + ln, l)
                        for l in range(4) for (t, off, ln) in self.out_chunks[l]]
        self.w2_blocks = []
        self.w2_block_of = {}
        for (t, ra, rb, ca, cb, l) in self.g2_grps:
            if (t, l) not in self.w2_block_of:
                self.w2_block_of[(t, l)] = len(self.w2_blocks)
                self.w2_blocks.append((t, l))
        self.out_valid = {}
        for (t, ra, rb, _, _, _) in self.g2_grps:
            self.out_valid[t] = max(self.out_valid.get(t, 0), rb)
        self.g2_by_tile = {}
        for (t, l) in self.w2_blocks:
            self.g2_by_tile.setdefault(t, []).append((l, self.w2_block_of[(t, l)]))

    def build_weights(self, w1, w2):
        """Masked per-(tile,block) weight column blocks, concatenated:
        W1m [w1_rows, 128 * len(w1_blocks)], W2m [w2_rows, 128 * len(w2_blocks)].
        Within a column block for input-block k, the rows of contraction-chunk
        ci sit at partition band ci*128 + off(k, ci) so the lhsT base partition
        matches the rhs activation chunk's partition offset."""
        P, Q, R = self.P, self.Q, self.R
        # unshifted: rows = local p / local r
        W1cols = np.zeros((P, self.mid_tiles * 128), np.float32)
        W2cols = np.zeros((R, self.out_tiles * 128), np.float32)
        for l in range(4):
            js = 4 * np.arange(R) + l
            ks, qs = js // Q, js % Q
            W1cols[:, cols_of_block(self.mid_chunks, l)] = np.ascontiguousarray(
                w1[ks, qs, :].T)
            W2cols[:, cols_of_block(self.out_chunks, l)] = np.ascontiguousarray(
                w2[l].T)
        W1m = np.zeros((self.w1_rows, 128 * len(self.w1_blocks)), np.float32)
        for (t, ra, rb, ca, cb, k) in self.g1_segs:
            bi = self.w1_block_of[(t, k)]
            p0 = 0
            for ci, (_, off, ln) in enumerate(self.in_chunks[k]):
                W1m[ci * 128 + off: ci * 128 + off + ln,
                    bi * 128 + ra: bi * 128 + rb] = W1cols[p0:p0 + ln, ca:cb]
                p0 += ln
        W2m = np.zeros((self.w2_rows, 128 * len(self.w2_blocks)), np.float32)
        for (t, ra, rb, ca, cb, l) in self.g2_grps:
            bi = self.w2_block_of[(t, l)]
            r0 = 0
            for ci, (_, off, ln) in enumerate(self.mid_chunks[l]):
                W2m[ci * 128 + off: ci * 128 + off + ln,
                    bi * 128 + ra: bi * 128 + rb] = W2cols[r0:r0 + ln, ca:cb]
                r0 += ln
        return W1m, W2m

    # -- l-groups (SBUF pressure): group g covers l in [2g, 2g+2) when ngroups==2
    def group_lset(self, g):
        return range(4) if self.ngroups == 1 else range(2 * g, 2 * g + 2)

    def group_midtiles(self, g):
        ls = set(self.group_lset(g))
        return sorted({t for l in ls for (t, _, _) in self.mid_chunks[l]})

    def group_outtiles(self, g):
        ls = set(self.group_lset(g))
        return sorted({t for l in ls for (t, _, _) in self.out_chunks[l]})


def build_plans():
    return [LayerPlan(i, s1, s2) for i, (s1, s2) in enumerate(SHAPES)]


# --------------------------------------------------- numpy model of the schedule
def numpy_forward(plans, weights, xT):
    """Execute the exact tiled/masked schedule in numpy (for validation).
    xT: [3072, B]. Returns log-probs [B, 10]."""
    B = xT.shape[1]
    h = np.zeros((plans[0].in_tiles * 128, B), np.float32)
    h[:xT.shape[0]] = xT
    for pl, (W1m, W2m) in zip(plans, weights):
        mid = np.zeros((pl.mid_tiles * 128, B), np.float32)
        for t, kbis in pl.g1_by_tile.items():
            V = pl.mid_valid[t]
            acc = np.zeros((V, B), np.float32)
            for (k, bi) in kbis:
                for ci, (t_in, off_in, ln_in) in enumerate(pl.in_chunks[k]):
                    lhsT = W1m[ci * 128 + off_in: ci * 128 + off_in + ln_in,
                               bi * 128: bi * 128 + V]
                    rhs = h[t_in * 128 + off_in: t_in * 128 + off_in + ln_in]
                    acc += lhsT.T @ rhs
            mid[t * 128: t * 128 + V] = acc
        out = np.zeros((pl.out_tiles * 128, B), np.float32)
        for t, lbis in pl.g2_by_tile.items():
            V = pl.out_valid[t]
            acc = np.zeros((V, B), np.float32)
            for (l, bi) in lbis:
                for ci, (t_m, off_m, ln_m) in enumerate(pl.mid_chunks[l]):
                    lhsT = W2m[ci * 128 + off_m: ci * 128 + off_m + ln_m,
                               bi * 128: bi * 128 + V]
                    rhs = mid[t_m * 128 + off_m: t_m * 128 + off_m + ln_m]
                    acc += lhsT.T @ rhs
            out[t * 128: t * 128 + V] = acc
        if pl.li < NLAYERS - 1:
            out = np.maximum(out, 0.0)
        h = out
    # final tile: rows l*32 + s (l in 0..3, s in 0..2); take first 10 features
    rows = np.array([l * 32 + s for l in range(4) for s in range(3)])[:NOUT]
    logits = h[rows, :].T
    m = logits.max(axis=1, keepdims=True)
    t = logits - m
    return t - np.log(np.exp(t).sum(axis=1, keepdims=True))


# ------------------------------------------------------------------ bass program
def build_program(plans):
    nc = bacc_mod.Bacc()

    xT = nc.dram_tensor("xT", [IN_FEATURES, BPC], ACT_DT, kind="ExternalInput")
    w1d = [nc.dram_tensor(f"w1c_{i}", [p.w1_rows, 128 * len(p.w1_blocks)], ACT_DT,
                          kind="ExternalInput") for i, p in enumerate(plans)]
    w2d = [nc.dram_tensor(f"w2c_{i}", [p.w2_rows, 128 * len(p.w2_blocks)], ACT_DT,
                          kind="ExternalInput") for i, p in enumerate(plans)]
    y = nc.dram_tensor("y", [BPC, NOUT], F32, kind="ExternalOutput")

    with tile.TileContext(nc) as tc:
        with (
            tc.tile_pool(name="dram", bufs=1, space="DRAM") as dpool,
            tc.tile_pool(name="sb", bufs=1) as sb,
            tc.tile_pool(name="ps", bufs=1, space="PSUM") as ps,
        ):
            # DRAM intermediates (output of layer li goes to hd[li])
            hd = []
            for li, pl in enumerate(plans[:-1]):
                hd.append(dpool.tile([pl.out_tiles * 128, BPC], ACT_DT,
                                     name=f"h_{li}", tag=f"h_{li}"))

            ident = sb.tile([128, 128], F32, name="ident", tag="ident")
            make_identity(nc, ident)

            evict_flip = [0]

            def evict(dst_ap, src_ap, relu):
                """PSUM -> SBUF eviction, alternating DVE/ACT to balance load."""
                e = evict_flip[0] = evict_flip[0] ^ 1
                if relu:
                    if e:
                        nc.vector.tensor_scalar_max(dst_ap, src_ap, 0.0)
                    else:
                        nc.scalar.activation(dst_ap, src_ap,
                                             mybir.ActivationFunctionType.Relu)
                else:
                    if e:
                        nc.vector.tensor_copy(dst_ap, src_ap)
                    else:
                        nc.scalar.copy(dst_ap, src_ap)

            trunc = int(_os.environ.get("KERNEL_TRUNC", str(NLAYERS)))
            for li, pl in enumerate(plans[:trunc]):
                hin_dram = xT if li == 0 else hd[li - 1]
                cpbm = pl.nmid_cpb
                for g in range(pl.ngroups):
                    mts = pl.group_midtiles(g)
                    ots = pl.group_outtiles(g)
                    mt_loc = {t: i for i, t in enumerate(mts)}
                    # weight blocks needed by this group
                    b1s = [bi for t in mts for (_, bi) in pl.g1_by_tile[t]]
                    b2s = [bi for t in ots for (_, bi) in pl.g2_by_tile[t]]
                    b1_loc = {bi: i for i, bi in enumerate(b1s)}
                    b2_loc = {bi: i for i, bi in enumerate(b2s)}
                    gmw = 128 * len(b1s)
                    gow = 128 * len(b2s)
                    # blocks of a group are contiguous in the dram layout
                    assert b1s == list(range(b1s[0], b1s[0] + len(b1s)))
                    assert b2s == list(range(b2s[0], b2s[0] + len(b2s)))
                    d1c0 = b1s[0] * 128
                    d2c0 = b2s[0] * 128

                    # ---- load weights (once per group, reused by both slices)
                    w1sb = sb.tile([128, pl.nin_cpb * gmw], ACT_DT,
                                   name=f"w1sb_{li}_{g}", tag="w1")
                    for c in range(pl.nin_cpb):
                        nc.sync.dma_start(
                            out=w1sb[:, c * gmw:(c + 1) * gmw],
                            in_=w1d[li][c * 128:(c + 1) * 128, d1c0:d1c0 + gmw])
                    w2sb = sb.tile([128, cpbm * gow], ACT_DT,
                                   name=f"w2sb_{li}_{g}", tag="w2")
                    for c in range(cpbm):
                        nc.sync.dma_start(
                            out=w2sb[:, c * gow:(c + 1) * gow],
                            in_=w2d[li][c * 128:(c + 1) * 128, d2c0:d2c0 + gow])

                    for sl in range(NSL):
                        c0 = sl * SLICE
                        # ---- load input activations for this slice
                        hin = sb.tile([128, pl.in_tiles * SLICE], ACT_DT,
                                      name=f"hin_{li}_{g}_{sl}", tag="hin")
                        DMA_GRP = 6
                        for t0 in range(0, pl.in_tiles, DMA_GRP):
                            t1 = min(t0 + DMA_GRP, pl.in_tiles)
                            src = hin_dram[t0 * 128: t1 * 128, c0:c0 + SLICE]
                            src = src.rearrange("(t p) n -> p t n", p=128)
                            dst = hin[:, t0 * SLICE: t1 * SLICE]
                            dst = dst.rearrange("p (t n) -> p t n", n=SLICE)
                            nc.sync.dma_start(out=dst, in_=src)

                        # ---- GEMM1: permuted-mid tiles
                        mid = sb.tile([128, len(mts) * SLICE], ACT_DT,
                                      name=f"mid_{li}_{g}_{sl}", tag="mid")
                        for t in mts:
                            V = pl.mid_valid[t]
                            pm = ps.tile([128, SLICE], F32, name=f"pm_{li}",
                                         tag="pmid", bufs=3)
                            chain = [(k, bi, ci, ch)
                                     for (k, bi) in pl.g1_by_tile[t]
                                     for ci, ch in enumerate(pl.in_chunks[k])]
                            for j, (k, bi, ci, (t_in, off_in, ln_in)) in \
                                    enumerate(chain):
                                lb = b1_loc[bi]
                                lhsT = w1sb[off_in:off_in + ln_in,
                                            ci * gmw + lb * 128:
                                            ci * gmw + lb * 128 + V]
                                rhs = hin[off_in:off_in + ln_in,
                                          t_in * SLICE:(t_in + 1) * SLICE]
                                nc.tensor.matmul(
                                    pm[0:V, :], lhsT, rhs,
                                    start=(j == 0), stop=(j == len(chain) - 1),
                                    tile_position=(off_in, 0))
                            loc = mt_loc[t]
                            evict(mid[0:V, loc * SLICE:(loc + 1) * SLICE],
                                  pm[0:V, :], relu=False)

                        # ---- GEMM2: block-diagonal, natural output order
                        if (li == trunc - 1 and trunc < NLAYERS
                                and _os.environ.get("KERNEL_ONLY_G1_LAST")):
                            continue
                        last = (li == NLAYERS - 1)
                        h7sb = None
                        if last:
                            h7sb = sb.tile([128, SLICE], F32,
                                           name=f"h7_{sl}", tag="h7")
                        for t in ots:
                            V = pl.out_valid[t]
                            po = ps.tile([128, SLICE], F32, name=f"po_{li}",
                                         tag="pout", bufs=3)
                            chain = [(l, bi, ci, ch)
                                     for (l, bi) in pl.g2_by_tile[t]
                                     for ci, ch in enumerate(pl.mid_chunks[l])]
                            for j, (l, bi, ci, (t_m, off_m, ln_m)) in \
                                    enumerate(chain):
                                lb = b2_loc[bi]
                                lhsT = w2sb[off_m:off_m + ln_m,
                                            ci * gow + lb * 128:
                                            ci * gow + lb * 128 + V]
                                loc = mt_loc[t_m]
                                rhs = mid[off_m:off_m + ln_m,
                                          loc * SLICE:(loc + 1) * SLICE]
                                nc.tensor.matmul(
                                    po[0:V, :], lhsT, rhs,
                                    start=(j == 0), stop=(j == len(chain) - 1),
                                    tile_position=(off_m, 0))
                            if last:
                                evict(h7sb[0:V, :], po[0:V, :], relu=False)
                            else:
                                hout = sb.tile([128, SLICE], ACT_DT,
                                               name=f"ho_{li}", tag="hout",
                                               bufs=4)
                                evict(hout[0:V, :], po[0:V, :], relu=True)
                                nc.sync.dma_start(
                                    out=hd[li][t * 128:t * 128 + V,
                                               c0:c0 + SLICE],
                                    in_=hout[0:V, :])

                        # ---- final log_softmax (h7 rows l*32+s; first 10)
                        if last:
                            for ch in range(SLICE // 128):
                                ptr = ps.tile([128, 128], F32, name="ptr",
                                              tag="ptr", bufs=2)
                                nc.tensor.transpose(
                                    ptr[:, 0:128],
                                    h7sb[0:128, ch * 128:(ch + 1) * 128],
                                    ident[0:128, 0:128])
                                # compact the 12 scattered cols (l*32+s) -> 12
                                cmp = sb.tile([128, 12], F32, name="cmp",
                                              tag="cmp", bufs=2)
                                src3 = ptr[:, 0:128].rearrange(
                                    "p (l o) -> p l o", l=4)[:, :, 0:3]
                                nc.vector.tensor_copy(
                                    cmp.rearrange("p (l o) -> p l o", l=4),
                                    src3)
                                mx = sb.tile([128, 1], F32, name="mx",
                                             tag="mx", bufs=2)
                                nc.vector.reduce_max(
                                    mx, cmp[:, 0:NOUT],
                                    axis=mybir.AxisListType.X)
                                tsb = sb.tile([128, NOUT], F32, name="tsb",
                                              tag="tsb", bufs=2)
                                nc.vector.tensor_scalar_sub(
                                    tsb, cmp[:, 0:NOUT], mx)
                                esb = sb.tile([128, NOUT], F32, name="esb",
                                              tag="esb", bufs=2)
                                esum = sb.tile([128, 1], F32, name="esum",
                                               tag="esum", bufs=2)
                                nc.scalar.activation(
                                    esb, tsb, mybir.ActivationFunctionType.Exp,
                                    accum_out=esum)
                                lse = sb.tile([128, 1], F32, name="lse",
                                              tag="lse", bufs=2)
                                nc.scalar.activation(
                                    lse, esum, mybir.ActivationFunctionType.Ln)
                                osb = sb.tile([128, NOUT], F32, name="osb",
                                              tag="osb", bufs=2)
                                nc.vector.tensor_scalar_sub(osb, tsb, lse)
                                r0 = sl * SLICE + ch * 128
                                nc.sync.dma_start(out=y[r0:r0 + 128, :],
                                                  in_=osb)
    nc.finalize()
    return nc


# ------------------------------------------------------------------ entry point
def _prep_inputs(inputs, plans):
    np_dt = mybir.dt.np(ACT_DT)
    x = np.ascontiguousarray(np.asarray(inputs["x"], dtype=np.float32))
    shared = {}
    for i, pl in enumerate(plans):
        w1 = np.asarray(inputs[f"w1_{i + 1}"], dtype=np.float32)
        w2 = np.asarray(inputs[f"w2_{i + 1}"], dtype=np.float32)
        W1m, W2m = pl.build_weights(w1, w2)
        shared[f"w1c_{i}"] = np.ascontiguousarray(W1m.astype(np_dt))
        shared[f"w2c_{i}"] = np.ascontiguousarray(W2m.astype(np_dt))
    in_maps = []
    for c in range(NCORES):
        m = dict(shared)
        m["xT"] = np.ascontiguousarray(x[c * BPC:(c + 1) * BPC].T.astype(np_dt))
        in_maps.append(m)
    return in_maps


def _run(inputs, trace=False, **spmd_kwargs):
    plans = build_plans()
    in_maps = _prep_inputs(inputs, plans)
    nc = build_program(plans)
    res = run_bass_kernel_spmd(nc, in_maps, core_ids=list(range(NCORES)),
                               trace=trace, **spmd_kwargs)
    out = np.concatenate([r["y"] for r in res.results], axis=0)
    return out.astype(np.float32), res


def kernel(**inputs):
    out, _ = _run(inputs, trace=False)
    return out


# revision 18
# speedup vs baseline: 1.2968x; 1.2968x over previous
"""Trainium2 Bass kernel for the CIFAR10 Monarch MLP (7 monarch layers + log_softmax).

Strategy
--------
Pure data parallel over 8 NeuronCores: each core takes a 1024-row batch shard;
the ~9M-param block-diagonal weights are replicated.

On-device dataflow is feature-major: activations live in SBUF as
[features (128-partition tiles), batch (free dim)].  The monarch butterfly
permutation is folded entirely into a host-side rearrangement of the weights:

  GEMM1:  W1cols[p_local, midcol(l,r)] = w1[k, q, p]   with j=4r+l, k=j//Q, q=j%Q
  GEMM2:  W2cols[r_local, outcol(l,s)] = w2[l, s, r]

so both GEMMs become plain matmuls over contiguous feature tiles; no on-device
transpose or permutation is needed anywhere except one tiny 12xN PE transpose
feeding the final log_softmax.

Matmul outputs must start at a 32-aligned PSUM partition, so each 128-row
output tile is produced by a single accumulation chain covering rows [0, valid)
with per-(tile, input-block) column-masked weight slices built on the host
(masking costs nothing on the PE: matmul time scales with N, not M).

Layers run one at a time over the full per-core batch (2 slices of 512
columns); intermediate activations round-trip DRAM between layers.  Matmuls
use the fp32r PE path (fp32 storage, full rate at N=512).
"""

import numpy as np

import concourse.bass as bass
import concourse.bacc as bacc_mod
import concourse.mybir as mybir
import concourse.tile as tile
from concourse.bass_utils import run_bass_kernel_spmd
from concourse.masks import make_identity

# ----------------------------------------------------------------- problem dims
BATCH = 8192
IN_FEATURES = 3072
NCORES = 8
BPC = BATCH // NCORES          # 1024 batch rows per core
NSL = 2                        # batch slices per core
SLICE = BPC // NSL             # 512
NOUT = 10

SHAPES = [((4, 750, 768), (4, 750, 750)),
          ((4, 500, 750), (4, 500, 500)),
          ((4, 250, 500), (4, 250, 250)),
          ((4, 125, 250), (4, 125, 125)),
          ((4, 50, 125), (4, 50, 50)),
          ((4, 25, 50), (4, 25, 25)),
          ((4, 3, 25), (4, 3, 3))]
NLAYERS = 7

F32 = mybir.dt.float32
# Activation/weight chain dtype for the two GEMMs: "fp32" (exact, 4 cyc/row),
# "fp32r" (fp32 storage, ~tf32 matmul precision, 1 cyc/row at N=512), or
# "bf16" (half storage/traffic, 1 cyc/row).
import os as _os
ACT_DT_NAME = _os.environ.get("KERNEL_MM_DT", "bf16")
ACT_DT = {"fp32": mybir.dt.float32,
          "fp32r": mybir.dt.float32r,
          "bf16": mybir.dt.bfloat16}[ACT_DT_NAME]


# ------------------------------------------------------------------ layouts
def make_layout(Sb, pack=False):
    """4 blocks of size Sb in 128-row tiles. Non-final layouts give every
    block its own tile at offset 0 (HW: accumulation chains cannot mix
    tile_position row bases, so contraction operands must sit at base 0).
    pack=True (final-layer output only) packs blocks at 32-aligned offsets
    inside one tile; those offsets only appear on the masked M side.
    Returns (ntiles, chunks), chunks[k] = [(tile, off, ln), ...]"""
    if Sb >= 128:
        cpb = (Sb + 127) // 128
        chunks = []
        for k in range(4):
            ck, rem, c = [], Sb, 0
            while rem > 0:
                ln = min(128, rem)
                ck.append((k * cpb + c, 0, ln))
                rem -= ln
                c += 1
            chunks.append(ck)
        return 4 * cpb, chunks
    if pack:
        stride = ((Sb + 31) // 32) * 32
        bpt = max(1, 128 // stride)
        ntiles = (4 + bpt - 1) // bpt
        return ntiles, [[(k // bpt, (k % bpt) * stride, Sb)] for k in range(4)]
    return 4, [[(k, 0, Sb)] for k in range(4)]


def cols_of_block(chunks, k):
    out = []
    for (t, off, ln) in chunks[k]:
        out.extend(range(t * 128 + off, t * 128 + off + ln))
    return np.array(out, dtype=np.int64)


class LayerPlan:
    def __init__(self, li, w1_shape, w2_shape):
        _, Q, P = w1_shape
        _, S, R = w2_shape
        self.li, self.P, self.Q, self.R, self.S = li, P, Q, R, S
        self.in_tiles, self.in_chunks = make_layout(P)
        self.mid_tiles, self.mid_chunks = make_layout(R)
        self.out_tiles, self.out_chunks = make_layout(S, pack=(li == NLAYERS - 1))
        self.nin_cpb = len(self.in_chunks[0])
        self.nmid_cpb = len(self.mid_chunks[0])
        self.w1_rows = self.nin_cpb * 128
        self.w2_rows = self.nmid_cpb * 128
        self.ngroups = 2 if li <= 1 else 1
        self._build_schedules()

    def _build_schedules(self):
        # g1 segments (pre-masking): (mid_tile, row_a, row_b, colA, colB, k)
        segs = []
        Q = self.Q
        for l in range(4):
            r0 = 0
            for (t, off, ln) in self.mid_chunks[l]:
                rr = np.arange(r0, r0 + ln)
                ks = (4 * rr + l) // Q
                a = 0
                while a < ln:
                    k, b = ks[a], a
                    while b < ln and ks[b] == k:
                        b += 1
                    segs.append((t, off + a, off + b,
                                 t * 128 + off + a, t * 128 + off + b, int(k)))
                    a = b
                r0 += ln
        self.g1_segs = segs
        # masked weight-1 blocks: one [w1_rows, 128] column block per (tile, k)
        self.w1_blocks = []        # list of (tile, k)
        self.w1_block_of = {}      # (tile, k) -> index
        for (t, ra, rb, ca, cb, k) in segs:
            if (t, k) not in self.w1_block_of:
                self.w1_block_of[(t, k)] = len(self.w1_blocks)
                self.w1_blocks.append((t, k))
        self.mid_valid = {}
        for (t, ra, rb, _, _, _) in segs:
            self.mid_valid[t] = max(self.mid_valid.get(t, 0), rb)
        # g1 schedule per mid tile: ordered (k, block_idx)
        self.g1_by_tile = {}
        for (t, k) in self.w1_blocks:
            self.g1_by_tile.setdefault(t, []).append((k, self.w1_block_of[(t, k)]))

        # g2 groups: (out_tile, row_a, row_b, ocolA, ocolB, l)
        self.g2_grps = [(t, off, off + ln, t * 128 + off, t * 128 + off + ln, l)
                        for l in range(4) for (t, off, ln) in self.out_chunks[l]]
        self.w2_blocks = []
        self.w2_block_of = {}
        for (t, ra, rb, ca, cb, l) in self.g2_grps:
            if (t, l) not in self.w2_block_of:
                self.w2_block_of[(t, l)] = len(self.w2_blocks)
                self.w2_blocks.append((t, l))
        self.out_valid = {}
        for (t, ra, rb, _, _, _) in self.g2_grps:
            self.out_valid[t] = max(self.out_valid.get(t, 0), rb)
        self.g2_by_tile = {}
        for (t, l) in self.w2_blocks:
            self.g2_by_tile.setdefault(t, []).append((l, self.w2_block_of[(t, l)]))

    def build_weights(self, w1, w2):
        """Masked per-(tile,block) weight column blocks, concatenated:
        W1m [w1_rows, 128 * len(w1_blocks)], W2m [w2_rows, 128 * len(w2_blocks)].
        Within a column block for input-block k, the rows of contraction-chunk
        ci sit at partition band ci*128 + off(k, ci) so the lhsT base partition
        matches the rhs activation chunk's partition offset."""
        P, Q, R = self.P, self.Q, self.R
        # unshifted: rows = local p / local r
        W1cols = np.zeros((P, self.mid_tiles * 128), np.float32)
        W2cols = np.zeros((R, self.out_tiles * 128), np.float32)
        for l in range(4):
            js = 4 * np.arange(R) + l
            ks, qs = js // Q, js % Q
            W1cols[:, cols_of_block(self.mid_chunks, l)] = np.ascontiguousarray(
                w1[ks, qs, :].T)
            W2cols[:, cols_of_block(self.out_chunks, l)] = np.ascontiguousarray(
                w2[l].T)
        W1m = np.zeros((self.w1_rows, 128 * len(self.w1_blocks)), np.float32)
        for (t, ra, rb, ca, cb, k) in self.g1_segs:
            bi = self.w1_block_of[(t, k)]
            p0 = 0
            for ci, (_, off, ln) in enumerate(self.in_chunks[k]):
                W1m[ci * 128 + off: ci * 128 + off + ln,
                    bi * 128 + ra: bi * 128 + rb] = W1cols[p0:p0 + ln, ca:cb]
                p0 += ln
        W2m = np.zeros((self.w2_rows, 128 * len(self.w2_blocks)), np.float32)
        for (t, ra, rb, ca, cb, l) in self.g2_grps:
            bi = self.w2_block_of[(t, l)]
            r0 = 0
            for ci, (_, off, ln) in enumerate(self.mid_chunks[l]):
                W2m[ci * 128 + off: ci * 128 + off + ln,
                    bi * 128 + ra: bi * 128 + rb] = W2cols[r0:r0 + ln, ca:cb]
                r0 += ln
        return W1m, W2m

    # -- l-groups (SBUF pressure): group g covers l in [2g, 2g+2) when ngroups==2
    def group_lset(self, g):
        return range(4) if self.ngroups == 1 else range(2 * g, 2 * g + 2)

    def group_midtiles(self, g):
        ls = set(self.group_lset(g))
        return sorted({t for l in ls for (t, _, _) in self.mid_chunks[l]})

    def group_outtiles(self, g):
        ls = set(self.group_lset(g))
        return sorted({t for l in ls for (t, _, _) in self.out_chunks[l]})


def build_plans():
    return [LayerPlan(i, s1, s2) for i, (s1, s2) in enumerate(SHAPES)]


# --------------------------------------------------- numpy model of the schedule
def numpy_forward(plans, weights, xT):
    """Execute the exact tiled/masked schedule in numpy (for validation).
    xT: [3072, B]. Returns log-probs [B, 10]."""
    B = xT.shape[1]
    h = np.zeros((plans[0].in_tiles * 128, B), np.float32)
    h[:xT.shape[0]] = xT
    for pl, (W1m, W2m) in zip(plans, weights):
        mid = np.zeros((pl.mid_tiles * 128, B), np.float32)
        for t, kbis in pl.g1_by_tile.items():
            V = pl.mid_valid[t]
            acc = np.zeros((V, B), np.float32)
            for (k, bi) in kbis:
                for ci, (t_in, off_in, ln_in) in enumerate(pl.in_chunks[k]):
                    lhsT = W1m[ci * 128 + off_in: ci * 128 + off_in + ln_in,
                               bi * 128: bi * 128 + V]
                    rhs = h[t_in * 128 + off_in: t_in * 128 + off_in + ln_in]
                    acc += lhsT.T @ rhs
            mid[t * 128: t * 128 + V] = acc
        out = np.zeros((pl.out_tiles * 128, B), np.float32)
        for t, lbis in pl.g2_by_tile.items():
            V = pl.out_valid[t]
            acc = np.zeros((V, B), np.float32)
            for (l, bi) in lbis:
                for ci, (t_m, off_m, ln_m) in enumerate(pl.mid_chunks[l]):
                    lhsT = W2m[ci * 128 + off_m: ci * 128 + off_m + ln_m,
                               bi * 128: bi * 128 + V]
                    rhs = mid[t_m * 128 + off_m: t_m * 128 + off_m + ln_m]
                    acc += lhsT.T @ rhs
            out[t * 128: t * 128 + V] = acc
        if pl.li < NLAYERS - 1:
            out = np.maximum(out, 0.0)
        h = out
    # final tile: rows l*32 + s (l in 0..3, s in 0..2); take first 10 features
    rows = np.array([l * 32 + s for l in range(4) for s in range(3)])[:NOUT]
    logits = h[rows, :].T
    m = logits.max(axis=1, keepdims=True)
    t = logits - m
    return t - np.log(np.exp(t).sum(axis=1, keepdims=True))


# ------------------------------------------------------------------ bass program
def build_program(plans):
    nc = bacc_mod.Bacc()

    xT = nc.dram_tensor("xT", [plans[0].in_tiles, 128, BPC], ACT_DT,
                        kind="ExternalInput")
    w1d = [nc.dram_tensor(f"w1c_{i}", [p.w1_rows, 128 * len(p.w1_blocks)], ACT_DT,
                          kind="ExternalInput") for i, p in enumerate(plans)]
    w2d = [nc.dram_tensor(f"w2c_{i}", [p.w2_rows, 128 * len(p.w2_blocks)], ACT_DT,
                          kind="ExternalInput") for i, p in enumerate(plans)]
    y = nc.dram_tensor("y", [BPC, NOUT], F32, kind="ExternalOutput")

    with tile.TileContext(nc) as tc:
        with (
            tc.tile_pool(name="sb", bufs=1) as sb,
            tc.tile_pool(name="ps", bufs=1, space="PSUM") as ps,
        ):
            ident = sb.tile([128, 128], F32, name="ident", tag="ident")
            make_identity(nc, ident)

            evict_flip = [0]

            def evict(dst_ap, src_ap, relu):
                """PSUM -> SBUF eviction, alternating DVE/ACT to balance load."""
                e = evict_flip[0] = evict_flip[0] ^ 1
                if relu:
                    if e:
                        nc.vector.tensor_scalar_max(dst_ap, src_ap, 0.0)
                    else:
                        nc.scalar.activation(dst_ap, src_ap,
                                             mybir.ActivationFunctionType.Relu)
                else:
                    if e:
                        nc.vector.tensor_copy(dst_ap, src_ap)
                    else:
                        nc.scalar.copy(dst_ap, src_ap)

            # ---- input activations, SBUF-resident ping-pong across layers
            hin = sb.tile([128, plans[0].in_tiles, BPC], ACT_DT,
                          name="h_in0", tag="hA")
            DMA_GRP = 6
            for t0 in range(0, plans[0].in_tiles, DMA_GRP):
                t1 = min(t0 + DMA_GRP, plans[0].in_tiles)
                nc.sync.dma_start(
                    out=hin[:, t0:t1, :],
                    in_=xT[t0:t1].rearrange("t p n -> p t n"))

            trunc = int(_os.environ.get("KERNEL_TRUNC", str(NLAYERS)))
            for li, pl in enumerate(plans[:trunc]):
                last = (li == NLAYERS - 1)
                cpbm = pl.nmid_cpb
                if last:
                    hnext = None
                    h7sb = sb.tile([128, BPC], F32, name="h7", tag="h7")
                else:
                    hnext = sb.tile([128, pl.out_tiles, BPC], ACT_DT,
                                    name=f"h_{li + 1}",
                                    tag="hB" if li % 2 == 0 else "hA")

                def g1_block(l, w1sb, gmw, b1_loc):
                    """GEMM1 for mid-block l -> fresh mid tile [128, cpb, BPC]."""
                    midl = sb.tile([128, cpbm, BPC], ACT_DT, name=f"mid_{li}_{l}",
                                   tag="midf" if last else "midb",
                                   bufs=5 if last else 2)
                    for ci_m, (t, _, ln_t) in enumerate(pl.mid_chunks[l]):
                        V = pl.mid_valid[t]
                        for cs in range(2):
                            c0 = cs * 512
                            pm = ps.tile([128, 512], F32, name=f"pm_{li}",
                                         tag="pmid", bufs=3)
                            chain = [(k, bi, ci, ch)
                                     for (k, bi) in pl.g1_by_tile[t]
                                     for ci, ch in enumerate(pl.in_chunks[k])]
                            for j, (k, bi, ci, (t_in, off_in, ln_in)) in \
                                    enumerate(chain):
                                lb = b1_loc[bi]
                                lhsT = w1sb[0:ln_in,
                                            ci * gmw + lb * 128:
                                            ci * gmw + lb * 128 + V]
                                rhs = hin[0:ln_in, t_in, c0:c0 + 512]
                                nc.tensor.matmul(
                                    pm[0:V, :], lhsT, rhs,
                                    start=(j == 0),
                                    stop=(j == len(chain) - 1))
                            evict(midl[0:V, ci_m, c0:c0 + 512], pm[0:V, :],
                                  relu=False)
                    return midl

                def g2_block(l, midl, w2sb, gow, b2_loc):
                    """GEMM2 for out tiles of block l (non-last layers)."""
                    for (t, _, _) in pl.out_chunks[l]:
                        V = pl.out_valid[t]
                        for cs in range(2):
                            c0 = cs * 512
                            po = ps.tile([128, 512], F32, name=f"po_{li}",
                                         tag="pout", bufs=3)
                            chain = [(bi, ci, ch)
                                     for (ll, bi) in pl.g2_by_tile[t]
                                     for ci, ch in enumerate(pl.mid_chunks[ll])]
                            for j, (bi, ci, (t_m, off_m, ln_m)) in \
                                    enumerate(chain):
                                lb = b2_loc[bi]
                                lhsT = w2sb[0:ln_m,
                                            ci * gow + lb * 128:
                                            ci * gow + lb * 128 + V]
                                rhs = midl[0:ln_m, ci, c0:c0 + 512]
                                nc.tensor.matmul(
                                    po[0:V, :], lhsT, rhs,
                                    start=(j == 0),
                                    stop=(j == len(chain) - 1))
                            evict(hnext[0:V, t, c0:c0 + 512], po[0:V, :],
                                  relu=True)

                for g in range(pl.ngroups):
                    ls = list(pl.group_lset(g))
                    mts = pl.group_midtiles(g)
                    ots = pl.group_outtiles(g)
                    b1s = [bi for t in mts for (_, bi) in pl.g1_by_tile[t]]
                    b2s = [bi for t in ots for (_, bi) in pl.g2_by_tile[t]]
                    b2s = sorted(set(b2s))
                    b1_loc = {bi: i for i, bi in enumerate(b1s)}
                    b2_loc = {bi: i for i, bi in enumerate(b2s)}
                    gmw = 128 * len(b1s)
                    gow = 128 * len(b2s)
                    assert b1s == list(range(b1s[0], b1s[0] + len(b1s)))
                    assert b2s == list(range(b2s[0], b2s[0] + len(b2s)))
                    d1c0 = b1s[0] * 128
                    d2c0 = b2s[0] * 128

                    w1sb = sb.tile([128, pl.nin_cpb * gmw], ACT_DT,
                                   name=f"w1sb_{li}_{g}", tag="w1")
                    for c in range(pl.nin_cpb):
                        nc.sync.dma_start(
                            out=w1sb[:, c * gmw:(c + 1) * gmw],
                            in_=w1d[li][c * 128:(c + 1) * 128, d1c0:d1c0 + gmw])
                    w2sb = sb.tile([128, cpbm * gow], ACT_DT,
                                   name=f"w2sb_{li}_{g}", tag="w2")
                    for c in range(cpbm):
                        nc.sync.dma_start(
                            out=w2sb[:, c * gow:(c + 1) * gow],
                            in_=w2d[li][c * 128:(c + 1) * 128, d2c0:d2c0 + gow])

                    if last:
                        # all four mid blocks feed the single packed out tile
                        mids = {l: g1_block(l, w1sb, gmw, b1_loc) for l in ls}
                        t = 0
                        V = pl.out_valid[t]
                        for cs in range(2):
                            c0 = cs * 512
                            po = ps.tile([128, 512], F32, name="po_f",
                                         tag="pout", bufs=3)
                            chain = [(ll, bi, ci, ch)
                                     for (ll, bi) in pl.g2_by_tile[t]
                                     for ci, ch in enumerate(pl.mid_chunks[ll])]
                            for j, (ll, bi, ci, (t_m, off_m, ln_m)) in \
                                    enumerate(chain):
                                lb = b2_loc[bi]
                                lhsT = w2sb[0:ln_m,
                                            ci * gow + lb * 128:
                                            ci * gow + lb * 128 + V]
                                rhs = mids[ll][0:ln_m, ci, c0:c0 + 512]
                                nc.tensor.matmul(
                                    po[0:V, :], lhsT, rhs,
                                    start=(j == 0),
                                    stop=(j == len(chain) - 1))
                            evict(h7sb[0:V, c0:c0 + 512], po[0:V, :],
                                  relu=False)
                    else:
                        # one-block lookahead: G1(l+1) is emitted before G2(l)
                        pend = None
                        for l in ls:
                            midl = g1_block(l, w1sb, gmw, b1_loc)
                            if pend is not None:
                                g2_block(pend[0], pend[1], w2sb, gow, b2_loc)
                            pend = (l, midl)
                        g2_block(pend[0], pend[1], w2sb, gow, b2_loc)

                if not last:
                    hin = hnext

            # ---- final log_softmax (h7 rows l*32+s hold the 12 logits)
            VF = plans[-1].out_valid[0]
            for ch in range(BPC // 128):
                ptr = ps.tile([128, 128], F32, name="ptr", tag="ptr", bufs=2)
                nc.tensor.transpose(
                    ptr[:, 0:VF],
                    h7sb[0:VF, ch * 128:(ch + 1) * 128],
                    ident[0:VF, 0:VF])
                cmp = sb.tile([128, 12], F32, name="cmp", tag="cmp", bufs=2)
                src3 = ptr[:, 0:128].rearrange("p (l o) -> p l o", l=4)[:, :, 0:3]
                nc.vector.tensor_copy(
                    cmp.rearrange("p (l o) -> p l o", l=4), src3)
                mx = sb.tile([128, 1], F32, name="mx", tag="mx", bufs=2)
                nc.vector.reduce_max(mx, cmp[:, 0:NOUT],
                                     axis=mybir.AxisListType.X)
                tsb = sb.tile([128, NOUT], F32, name="tsb", tag="tsb", bufs=2)
                nc.vector.tensor_scalar_sub(tsb, cmp[:, 0:NOUT], mx)
                esb = sb.tile([128, NOUT], F32, name="esb", tag="esb", bufs=2)
                esum = sb.tile([128, 1], F32, name="esum", tag="esum", bufs=2)
                nc.scalar.activation(esb, tsb,
                                     mybir.ActivationFunctionType.Exp,
                                     accum_out=esum)
                lse = sb.tile([128, 1], F32, name="lse", tag="lse", bufs=2)
                nc.scalar.activation(lse, esum,
                                     mybir.ActivationFunctionType.Ln)
                osb = sb.tile([128, NOUT], F32, name="osb", tag="osb", bufs=2)
                nc.vector.tensor_scalar_sub(osb, tsb, lse)
                nc.sync.dma_start(out=y[ch * 128:(ch + 1) * 128, :], in_=osb)
    nc.finalize()
    return nc


# ------------------------------------------------------------------ entry point
def _prep_inputs(inputs, plans):
    np_dt = mybir.dt.np(ACT_DT)
    x = np.ascontiguousarray(np.asarray(inputs["x"], dtype=np.float32))
    shared = {}
    for i, pl in enumerate(plans):
        w1 = np.asarray(inputs[f"w1_{i + 1}"], dtype=np.float32)
        w2 = np.asarray(inputs[f"w2_{i + 1}"], dtype=np.float32)
        W1m, W2m = pl.build_weights(w1, w2)
        shared[f"w1c_{i}"] = np.ascontiguousarray(W1m.astype(np_dt))
        shared[f"w2c_{i}"] = np.ascontiguousarray(W2m.astype(np_dt))
    in_maps = []
    for c in range(NCORES):
        m = dict(shared)
        m["xT"] = np.ascontiguousarray(x[c * BPC:(c + 1) * BPC].T.astype(np_dt))
        in_maps.append(m)
    return in_maps


def _run(inputs, trace=False, **spmd_kwargs):
    plans = build_plans()
    in_maps = _prep_inputs(inputs, plans)
    nc = build_program(plans)
    res = run_bass_kernel_spmd(nc, in_maps, core_ids=list(range(NCORES)),
                               trace=trace, **spmd_kwargs)
    out = np.concatenate([r["y"] for r in res.results], axis=0)
    return out.astype(np.float32), res


def kernel(**inputs):
    out, _ = _run(inputs, trace=False)
    return out


# revision 22
# speedup vs baseline: 1.4715x; 1.1347x over previous
"""Trainium2 Bass kernel for the CIFAR10 Monarch MLP (7 monarch layers + log_softmax).

Strategy
--------
Pure data parallel over 8 NeuronCores: each core takes a 1024-row batch shard;
the ~9M-param block-diagonal weights are replicated.

On-device dataflow is feature-major: activations live in SBUF as
[features (128-partition tiles), batch (free dim)].  The monarch butterfly
permutation is folded entirely into a host-side rearrangement of the weights:

  GEMM1:  W1cols[p_local, midcol(l,r)] = w1[k, q, p]   with j=4r+l, k=j//Q, q=j%Q
  GEMM2:  W2cols[r_local, outcol(l,s)] = w2[l, s, r]

so both GEMMs become plain matmuls over contiguous feature tiles; no on-device
transpose or permutation is needed anywhere except one tiny 12xN PE transpose
feeding the final log_softmax.

Matmul outputs must start at a 32-aligned PSUM partition, so each 128-row
output tile is produced by a single accumulation chain covering rows [0, valid)
with per-(tile, input-block) column-masked weight slices built on the host
(masking costs nothing on the PE: matmul time scales with N, not M).

Layers run one at a time over the full per-core batch (2 slices of 512
columns); intermediate activations round-trip DRAM between layers.  Matmuls
use the fp32r PE path (fp32 storage, full rate at N=512).
"""

import numpy as np

import concourse.bass as bass
import concourse.bacc as bacc_mod
import concourse.mybir as mybir
import concourse.tile as tile
from concourse.bass_utils import run_bass_kernel_spmd
from concourse.masks import make_identity

# ----------------------------------------------------------------- problem dims
BATCH = 8192
IN_FEATURES = 3072
NCORES = 8
BPC = BATCH // NCORES          # 1024 batch rows per core
NSL = 2                        # batch slices per core
SLICE = BPC // NSL             # 512
NOUT = 10

SHAPES = [((4, 750, 768), (4, 750, 750)),
          ((4, 500, 750), (4, 500, 500)),
          ((4, 250, 500), (4, 250, 250)),
          ((4, 125, 250), (4, 125, 125)),
          ((4, 50, 125), (4, 50, 50)),
          ((4, 25, 50), (4, 25, 25)),
          ((4, 3, 25), (4, 3, 3))]
NLAYERS = 7

F32 = mybir.dt.float32
# Activation/weight chain dtype for the two GEMMs: "fp32" (exact, 4 cyc/row),
# "fp32r" (fp32 storage, ~tf32 matmul precision, 1 cyc/row at N=512), or
# "bf16" (half storage/traffic, 1 cyc/row).
import os as _os
ACT_DT_NAME = _os.environ.get("KERNEL_MM_DT", "bf16")
ACT_DT = {"fp32": mybir.dt.float32,
          "fp32r": mybir.dt.float32r,
          "bf16": mybir.dt.bfloat16}[ACT_DT_NAME]


# ------------------------------------------------------------------ layouts
def make_layout(Sb, pack=False):
    """4 blocks of size Sb in 128-row tiles. Non-final layouts give every
    block its own tile at offset 0 (HW: accumulation chains cannot mix
    tile_position row bases, so contraction operands must sit at base 0).
    pack=True (final-layer output only) packs blocks at 32-aligned offsets
    inside one tile; those offsets only appear on the masked M side.
    Returns (ntiles, chunks), chunks[k] = [(tile, off, ln), ...]"""
    if Sb >= 128:
        cpb = (Sb + 127) // 128
        chunks = []
        for k in range(4):
            ck, rem, c = [], Sb, 0
            while rem > 0:
                ln = min(128, rem)
                ck.append((k * cpb + c, 0, ln))
                rem -= ln
                c += 1
            chunks.append(ck)
        return 4 * cpb, chunks
    if pack:
        stride = ((Sb + 31) // 32) * 32
        bpt = max(1, 128 // stride)
        ntiles = (4 + bpt - 1) // bpt
        return ntiles, [[(k // bpt, (k % bpt) * stride, Sb)] for k in range(4)]
    return 4, [[(k, 0, Sb)] for k in range(4)]


def cols_of_block(chunks, k):
    out = []
    for (t, off, ln) in chunks[k]:
        out.extend(range(t * 128 + off, t * 128 + off + ln))
    return np.array(out, dtype=np.int64)


class LayerPlan:
    def __init__(self, li, w1_shape, w2_shape):
        _, Q, P = w1_shape
        _, S, R = w2_shape
        self.li, self.P, self.Q, self.R, self.S = li, P, Q, R, S
        self.in_tiles, self.in_chunks = make_layout(P)
        self.mid_tiles, self.mid_chunks = make_layout(R)
        self.out_tiles, self.out_chunks = make_layout(S, pack=(li == NLAYERS - 1))
        self.nin_cpb = len(self.in_chunks[0])
        self.nmid_cpb = len(self.mid_chunks[0])
        self.w1_rows = self.nin_cpb * 128
        self.w2_rows = self.nmid_cpb * 128
        self.ngroups = 2 if li <= 1 else 1
        self._build_schedules()

    def _build_schedules(self):
        # Mid-space feature ordering. GEMM2's contraction order over r is
        # arbitrary (W2 rows follow), so within each mid block l we reorder
        # features by their GEMM1 input block k — padded so k-groups align to
        # tile boundaries where possible — which removes most k-crossing
        # splits (each split costs a full extra accumulation chain).
        Q, R = self.Q, self.R
        self.mid_pos = []         # per l: array[R] -> position within block
        self.pos_k = []           # per l: array[block_rows] -> k or -1 (pad)
        block_rows = self.nmid_cpb * 128
        for l in range(4):
            rs = np.arange(R)
            ks = (4 * rs + l) // Q
            if R >= 125:
                # group by k, pad each group to Gp = block_rows // 4
                Gp = block_rows // 4
                pos = np.empty(R, np.int64)
                pk = np.full(block_rows, -1, np.int64)
                for k in range(4):
                    idx = rs[ks == k]
                    assert len(idx) <= Gp
                    pos[idx] = k * Gp + np.arange(len(idx))
                    pk[k * Gp: k * Gp + len(idx)] = k
            else:
                pos = rs.copy()
                pk = np.full(block_rows, -1, np.int64)
                pk[:R] = ks
            self.mid_pos.append(pos)
            self.pos_k.append(pk)

        # g1 segments: (mid_tile, row_a, row_b, k) — runs of constant k in
        # position order within each 128-row tile of each block.
        segs = []
        for l in range(4):
            pk = self.pos_k[l]
            base_t = self.mid_chunks[l][0][0]
            for c in range(self.nmid_cpb):
                t = base_t + c
                kk = pk[c * 128:(c + 1) * 128]
                a = 0
                while a < len(kk):
                    if kk[a] < 0:
                        a += 1
                        continue
                    k, b = kk[a], a
                    while b < len(kk) and kk[b] == k:
                        b += 1
                    segs.append((t, a, b, int(k)))
                    a = b
        self.g1_segs = segs
        # masked weight-1 blocks: one [w1_rows, 128] column block per (tile, k)
        self.w1_blocks = []        # list of (tile, k)
        self.w1_block_of = {}      # (tile, k) -> index
        for (t, ra, rb, k) in segs:
            if (t, k) not in self.w1_block_of:
                self.w1_block_of[(t, k)] = len(self.w1_blocks)
                self.w1_blocks.append((t, k))
        self.mid_valid = {}
        for (t, ra, rb, _) in segs:
            self.mid_valid[t] = max(self.mid_valid.get(t, 0), rb)
        # clamp mid chunk lengths to the written rows (trailing pads are
        # never produced, so GEMM2 must not read them)
        self.mid_chunks = [
            [(t, off, min(ln if self.R < 125 else 128, self.mid_valid[t]))
             for (t, off, ln) in self.mid_chunks[l]]
            for l in range(4)]
        # g1 schedule per mid tile: ordered (k, block_idx)
        self.g1_by_tile = {}
        for (t, k) in self.w1_blocks:
            self.g1_by_tile.setdefault(t, []).append((k, self.w1_block_of[(t, k)]))

        # g2 groups: (out_tile, row_a, row_b, ocolA, ocolB, l)
        self.g2_grps = [(t, off, off + ln, t * 128 + off, t * 128 + off + ln, l)
                        for l in range(4) for (t, off, ln) in self.out_chunks[l]]
        self.w2_blocks = []
        self.w2_block_of = {}
        for (t, ra, rb, ca, cb, l) in self.g2_grps:
            if (t, l) not in self.w2_block_of:
                self.w2_block_of[(t, l)] = len(self.w2_blocks)
                self.w2_blocks.append((t, l))
        self.out_valid = {}
        for (t, ra, rb, _, _, _) in self.g2_grps:
            self.out_valid[t] = max(self.out_valid.get(t, 0), rb)
        self.g2_by_tile = {}
        for (t, l) in self.w2_blocks:
            self.g2_by_tile.setdefault(t, []).append((l, self.w2_block_of[(t, l)]))

    def build_weights(self, w1, w2):
        """Masked per-(tile,block) weight column blocks, concatenated:
        W1m [w1_rows, 128 * len(w1_blocks)], W2m [w2_rows, 128 * len(w2_blocks)].
        Within a column block for input-block k, the rows of contraction-chunk
        ci sit at partition band ci*128 + off(k, ci) so the lhsT base partition
        matches the rhs activation chunk's partition offset."""
        P, Q, R = self.P, self.Q, self.R
        block_rows = self.nmid_cpb * 128
        # unshifted: W1cols rows = local p, cols = mid position;
        #            W2cols rows = mid position within block, cols = out col
        W1cols = np.zeros((P, self.mid_tiles * 128), np.float32)
        W2cols = np.zeros((self.w2_rows, self.out_tiles * 128), np.float32)
        for l in range(4):
            js = 4 * np.arange(R) + l
            ks, qs = js // Q, js % Q
            base = self.mid_chunks[l][0][0] * 128
            mcols = base + self.mid_pos[l]
            W1cols[:, mcols] = np.ascontiguousarray(w1[ks, qs, :].T)
            ocols = cols_of_block(self.out_chunks, l)
            W2cols[np.ix_(self.mid_pos[l], ocols)] = np.ascontiguousarray(
                w2[l].T)
        W1m = np.zeros((self.w1_rows, 128 * len(self.w1_blocks)), np.float32)
        for (t, ra, rb, k) in self.g1_segs:
            bi = self.w1_block_of[(t, k)]
            ca, cb = t * 128 + ra, t * 128 + rb
            p0 = 0
            for ci, (_, off, ln) in enumerate(self.in_chunks[k]):
                W1m[ci * 128 + off: ci * 128 + off + ln,
                    bi * 128 + ra: bi * 128 + rb] = W1cols[p0:p0 + ln, ca:cb]
                p0 += ln
        W2m = np.zeros((self.w2_rows, 128 * len(self.w2_blocks)), np.float32)
        for (t, ra, rb, ca, cb, l) in self.g2_grps:
            bi = self.w2_block_of[(t, l)]
            for ci, (_, off, ln) in enumerate(self.mid_chunks[l]):
                W2m[ci * 128 + off: ci * 128 + off + ln,
                    bi * 128 + ra: bi * 128 + rb] = \
                    W2cols[ci * 128: ci * 128 + ln, ca:cb]
        return W1m, W2m

    # -- l-groups (SBUF pressure): group g covers l in [2g, 2g+2) when ngroups==2
    def group_lset(self, g):
        return range(4) if self.ngroups == 1 else range(2 * g, 2 * g + 2)

    def group_midtiles(self, g):
        ls = set(self.group_lset(g))
        return sorted({t for l in ls for (t, _, _) in self.mid_chunks[l]})

    def group_outtiles(self, g):
        ls = set(self.group_lset(g))
        return sorted({t for l in ls for (t, _, _) in self.out_chunks[l]})


def build_plans():
    return [LayerPlan(i, s1, s2) for i, (s1, s2) in enumerate(SHAPES)]


# --------------------------------------------------- numpy model of the schedule
def numpy_forward(plans, weights, xT):
    """Execute the exact tiled/masked schedule in numpy (for validation).
    xT: [3072, B]. Returns log-probs [B, 10]."""
    B = xT.shape[1]
    h = np.zeros((plans[0].in_tiles * 128, B), np.float32)
    h[:xT.shape[0]] = xT
    for pl, (W1m, W2m) in zip(plans, weights):
        mid = np.zeros((pl.mid_tiles * 128, B), np.float32)
        for t, kbis in pl.g1_by_tile.items():
            V = pl.mid_valid[t]
            acc = np.zeros((V, B), np.float32)
            for (k, bi) in kbis:
                for ci, (t_in, off_in, ln_in) in enumerate(pl.in_chunks[k]):
                    lhsT = W1m[ci * 128 + off_in: ci * 128 + off_in + ln_in,
                               bi * 128: bi * 128 + V]
                    rhs = h[t_in * 128 + off_in: t_in * 128 + off_in + ln_in]
                    acc += lhsT.T @ rhs
            mid[t * 128: t * 128 + V] = acc
        out = np.zeros((pl.out_tiles * 128, B), np.float32)
        for t, lbis in pl.g2_by_tile.items():
            V = pl.out_valid[t]
            acc = np.zeros((V, B), np.float32)
            for (l, bi) in lbis:
                for ci, (t_m, off_m, ln_m) in enumerate(pl.mid_chunks[l]):
                    lhsT = W2m[ci * 128 + off_m: ci * 128 + off_m + ln_m,
                               bi * 128: bi * 128 + V]
                    rhs = mid[t_m * 128 + off_m: t_m * 128 + off_m + ln_m]
                    acc += lhsT.T @ rhs
            out[t * 128: t * 128 + V] = acc
        if pl.li < NLAYERS - 1:
            out = np.maximum(out, 0.0)
        h = out
    # final tile: rows l*32 + s (l in 0..3, s in 0..2); take first 10 features
    rows = np.array([l * 32 + s for l in range(4) for s in range(3)])[:NOUT]
    logits = h[rows, :].T
    m = logits.max(axis=1, keepdims=True)
    t = logits - m
    return t - np.log(np.exp(t).sum(axis=1, keepdims=True))


# ------------------------------------------------------------------ bass program
def build_program(plans):
    nc = bacc_mod.Bacc()

    xT = nc.dram_tensor("xT", [plans[0].in_tiles, 128, BPC], ACT_DT,
                        kind="ExternalInput")
    w1d = [nc.dram_tensor(f"w1c_{i}", [p.w1_rows, 128 * len(p.w1_blocks)], ACT_DT,
                          kind="ExternalInput") for i, p in enumerate(plans)]
    w2d = [nc.dram_tensor(f"w2c_{i}", [p.w2_rows, 128 * len(p.w2_blocks)], ACT_DT,
                          kind="ExternalInput") for i, p in enumerate(plans)]
    y = nc.dram_tensor("y", [BPC, NOUT], F32, kind="ExternalOutput")

    with tile.TileContext(nc) as tc:
        with (
            tc.tile_pool(name="sb", bufs=1) as sb,
            tc.tile_pool(name="ps", bufs=1, space="PSUM") as ps,
        ):
            ident = sb.tile([128, 128], F32, name="ident", tag="ident")
            make_identity(nc, ident)

            evict_flip = [0]

            def evict(dst_ap, src_ap, relu):
                """PSUM -> SBUF eviction, alternating DVE/ACT to balance load."""
                e = evict_flip[0] = evict_flip[0] ^ 1
                if relu:
                    if e:
                        nc.vector.tensor_scalar_max(dst_ap, src_ap, 0.0)
                    else:
                        nc.scalar.activation(dst_ap, src_ap,
                                             mybir.ActivationFunctionType.Relu)
                else:
                    if e:
                        nc.vector.tensor_copy(dst_ap, src_ap)
                    else:
                        nc.scalar.copy(dst_ap, src_ap)

            # ---- input activations, SBUF-resident ping-pong across layers
            hin = sb.tile([128, plans[0].in_tiles, BPC], ACT_DT,
                          name="h_in0", tag="hA")
            DMA_GRP = 6
            for t0 in range(0, plans[0].in_tiles, DMA_GRP):
                t1 = min(t0 + DMA_GRP, plans[0].in_tiles)
                nc.sync.dma_start(
                    out=hin[:, t0:t1, :],
                    in_=xT[t0:t1].rearrange("t p n -> p t n"))

            trunc = int(_os.environ.get("KERNEL_TRUNC", str(NLAYERS)))
            for li, pl in enumerate(plans[:trunc]):
                last = (li == NLAYERS - 1)
                cpbm = pl.nmid_cpb
                if last:
                    hnext = None
                    h7sb = sb.tile([128, BPC], F32, name="h7", tag="h7")
                else:
                    hnext = sb.tile([128, pl.out_tiles, BPC], ACT_DT,
                                    name=f"h_{li + 1}",
                                    tag="hB" if li % 2 == 0 else "hA")

                def g1_block(l, w1sb, gmw, b1_loc):
                    """GEMM1 for mid-block l -> fresh mid tile [128, cpb, BPC]."""
                    midl = sb.tile([128, cpbm, BPC], ACT_DT, name=f"mid_{li}_{l}",
                                   tag="midf" if last else "midb",
                                   bufs=5 if last else 2)
                    for ci_m, (t, _, ln_t) in enumerate(pl.mid_chunks[l]):
                        V = pl.mid_valid[t]
                        for cs in range(2):
                            c0 = cs * 512
                            pm = ps.tile([128, 512], F32, name=f"pm_{li}",
                                         tag="pmid", bufs=3)
                            chain = [(k, bi, ci, ch)
                                     for (k, bi) in pl.g1_by_tile[t]
                                     for ci, ch in enumerate(pl.in_chunks[k])]
                            for j, (k, bi, ci, (t_in, off_in, ln_in)) in \
                                    enumerate(chain):
                                lb = b1_loc[bi]
                                lhsT = w1sb[0:ln_in,
                                            ci * gmw + lb * 128:
                                            ci * gmw + lb * 128 + V]
                                rhs = hin[0:ln_in, t_in, c0:c0 + 512]
                                nc.tensor.matmul(
                                    pm[0:V, :], lhsT, rhs,
                                    start=(j == 0),
                                    stop=(j == len(chain) - 1))
                            evict(midl[0:V, ci_m, c0:c0 + 512], pm[0:V, :],
                                  relu=False)
                    return midl

                def g2_block(l, midl, w2sb, gow, b2_loc):
                    """GEMM2 for out tiles of block l (non-last layers)."""
                    for (t, _, _) in pl.out_chunks[l]:
                        V = pl.out_valid[t]
                        for cs in range(2):
                            c0 = cs * 512
                            po = ps.tile([128, 512], F32, name=f"po_{li}",
                                         tag="pout", bufs=3)
                            chain = [(bi, ci, ch)
                                     for (ll, bi) in pl.g2_by_tile[t]
                                     for ci, ch in enumerate(pl.mid_chunks[ll])]
                            for j, (bi, ci, (t_m, off_m, ln_m)) in \
                                    enumerate(chain):
                                lb = b2_loc[bi]
                                lhsT = w2sb[0:ln_m,
                                            ci * gow + lb * 128:
                                            ci * gow + lb * 128 + V]
                                rhs = midl[0:ln_m, ci, c0:c0 + 512]
                                nc.tensor.matmul(
                                    po[0:V, :], lhsT, rhs,
                                    start=(j == 0),
                                    stop=(j == len(chain) - 1))
                            evict(hnext[0:V, t, c0:c0 + 512], po[0:V, :],
                                  relu=True)

                for g in range(pl.ngroups):
                    ls = list(pl.group_lset(g))
                    mts = pl.group_midtiles(g)
                    ots = pl.group_outtiles(g)
                    b1s = [bi for t in mts for (_, bi) in pl.g1_by_tile[t]]
                    b2s = [bi for t in ots for (_, bi) in pl.g2_by_tile[t]]
                    b2s = sorted(set(b2s))
                    b1_loc = {bi: i for i, bi in enumerate(b1s)}
                    b2_loc = {bi: i for i, bi in enumerate(b2s)}
                    gmw = 128 * len(b1s)
                    gow = 128 * len(b2s)
                    assert b1s == list(range(b1s[0], b1s[0] + len(b1s)))
                    assert b2s == list(range(b2s[0], b2s[0] + len(b2s)))
                    d1c0 = b1s[0] * 128
                    d2c0 = b2s[0] * 128

                    w1sb = sb.tile([128, pl.nin_cpb * gmw], ACT_DT,
                                   name=f"w1sb_{li}_{g}", tag="w1")
                    for c in range(pl.nin_cpb):
                        nc.sync.dma_start(
                            out=w1sb[:, c * gmw:(c + 1) * gmw],
                            in_=w1d[li][c * 128:(c + 1) * 128, d1c0:d1c0 + gmw])
                    w2sb = sb.tile([128, cpbm * gow], ACT_DT,
                                   name=f"w2sb_{li}_{g}", tag="w2")
                    for c in range(cpbm):
                        nc.sync.dma_start(
                            out=w2sb[:, c * gow:(c + 1) * gow],
                            in_=w2d[li][c * 128:(c + 1) * 128, d2c0:d2c0 + gow])

                    if last:
                        # all four mid blocks feed the single packed out tile
                        mids = {l: g1_block(l, w1sb, gmw, b1_loc) for l in ls}
                        t = 0
                        V = pl.out_valid[t]
                        for cs in range(2):
                            c0 = cs * 512
                            po = ps.tile([128, 512], F32, name="po_f",
                                         tag="pout", bufs=3)
                            chain = [(ll, bi, ci, ch)
                                     for (ll, bi) in pl.g2_by_tile[t]
                                     for ci, ch in enumerate(pl.mid_chunks[ll])]
                            for j, (ll, bi, ci, (t_m, off_m, ln_m)) in \
                                    enumerate(chain):
                                lb = b2_loc[bi]
                                lhsT = w2sb[0:ln_m,
                                            ci * gow + lb * 128:
                                            ci * gow + lb * 128 + V]
                                rhs = mids[ll][0:ln_m, ci, c0:c0 + 512]
                                nc.tensor.matmul(
                                    po[0:V, :], lhsT, rhs,
                                    start=(j == 0),
                                    stop=(j == len(chain) - 1))
                            evict(h7sb[0:V, c0:c0 + 512], po[0:V, :],
                                  relu=False)
                    else:
                        # one-block lookahead: G1(l+1) is emitted before G2(l)
                        pend = None
                        for l in ls:
                            midl = g1_block(l, w1sb, gmw, b1_loc)
                            if pend is not None:
                                g2_block(pend[0], pend[1], w2sb, gow, b2_loc)
                            pend = (l, midl)
                        g2_block(pend[0], pend[1], w2sb, gow, b2_loc)

                if not last:
                    hin = hnext

            # ---- final log_softmax (h7 rows l*32+s hold the 12 logits)
            VF = plans[-1].out_valid[0]
            for ch in range(BPC // 128):
                ptr = ps.tile([128, 128], F32, name="ptr", tag="ptr", bufs=2)
                nc.tensor.transpose(
                    ptr[:, 0:VF],
                    h7sb[0:VF, ch * 128:(ch + 1) * 128],
                    ident[0:VF, 0:VF])
                cmp = sb.tile([128, 12], F32, name="cmp", tag="cmp", bufs=2)
                src3 = ptr[:, 0:128].rearrange("p (l o) -> p l o", l=4)[:, :, 0:3]
                nc.vector.tensor_copy(
                    cmp.rearrange("p (l o) -> p l o", l=4), src3)
                mx = sb.tile([128, 1], F32, name="mx", tag="mx", bufs=2)
                nc.vector.reduce_max(mx, cmp[:, 0:NOUT],
                                     axis=mybir.AxisListType.X)
                tsb = sb.tile([128, NOUT], F32, name="tsb", tag="tsb", bufs=2)
                nc.vector.tensor_scalar_sub(tsb, cmp[:, 0:NOUT], mx)
                esb = sb.tile([128, NOUT], F32, name="esb", tag="esb", bufs=2)
                esum = sb.tile([128, 1], F32, name="esum", tag="esum", bufs=2)
                nc.scalar.activation(esb, tsb,
                                     mybir.ActivationFunctionType.Exp,
                                     accum_out=esum)
                lse = sb.tile([128, 1], F32, name="lse", tag="lse", bufs=2)
                nc.scalar.activation(lse, esum,
                                     mybir.ActivationFunctionType.Ln)
                osb = sb.tile([128, NOUT], F32, name="osb", tag="osb", bufs=2)
                nc.vector.tensor_scalar_sub(osb, tsb, lse)
                nc.sync.dma_start(out=y[ch * 128:(ch + 1) * 128, :], in_=osb)
    nc.finalize()
    return nc


# ------------------------------------------------------------------ entry point
def _prep_inputs(inputs, plans):
    np_dt = mybir.dt.np(ACT_DT)
    x = np.ascontiguousarray(np.asarray(inputs["x"], dtype=np.float32))
    shared = {}
    for i, pl in enumerate(plans):
        w1 = np.asarray(inputs[f"w1_{i + 1}"], dtype=np.float32)
        w2 = np.asarray(inputs[f"w2_{i + 1}"], dtype=np.float32)
        W1m, W2m = pl.build_weights(w1, w2)
        shared[f"w1c_{i}"] = np.ascontiguousarray(W1m.astype(np_dt))
        shared[f"w2c_{i}"] = np.ascontiguousarray(W2m.astype(np_dt))
    in_maps = []
    for c in range(NCORES):
        m = dict(shared)
        m["xT"] = np.ascontiguousarray(x[c * BPC:(c + 1) * BPC].T.astype(np_dt))
        in_maps.append(m)
    return in_maps


def _run(inputs, trace=False, **spmd_kwargs):
    plans = build_plans()
    in_maps = _prep_inputs(inputs, plans)
    nc = build_program(plans)
    res = run_bass_kernel_spmd(nc, in_maps, core_ids=list(range(NCORES)),
                               trace=trace, **spmd_kwargs)
    out = np.concatenate([r["y"] for r in res.results], axis=0)
    return out.astype(np.float32), res


def kernel(**inputs):
    out, _ = _run(inputs, trace=False)
    return out
